# revision 41
# baseline (speedup 1.0000x reference)
import sys

import numpy as np

sys.path.insert(0, "/opt/trn_rl_repo")

from concourse import bacc, bass, tile  # noqa: E402,F401
from concourse import mybir  # noqa: E402
from concourse.bass import broadcast_tensor_aps  # noqa: E402
from concourse.bass_utils import run_bass_kernel_spmd  # noqa: E402

N_CORES = 8
S = 8  # samples per core
C = 3
T = 9
H = W = 256
RC = 8  # rows per chunk (one SBUF partition holds one chunk)
NCH = H // RC  # 32 chunks per sample
SPS = 128 // NCH  # 4 samples striped across the 128 partitions
NSTRIPES = S // SPS  # 2
RP = RC + 2  # rows per chunk incl halo (materialized on host)
WP = W + 2  # cols incl reflect pad (materialized on host)
HR = 4  # rows per mul tile (half a chunk)
BR = 2  # rows per PSUM block: 3ch acc + den = 4 banks, 2 blocks
BPX = BR * W  # 512 psum columns per block
F32 = mybir.dt.float32
F16 = mybir.dt.float16
# Per-half tap order: center tap first, then taps in merged-sigma-DMA
# arrival order {4}, {3,5}, {6,7,8}, {0,1,2}
TAPS_H0 = [4, 5, 3, 8, 7, 6, 2, 1, 0]
TAPS_H1 = [4, 5, 3, 2, 1, 0, 8, 7, 6]


def build_nc():
    nc = bacc.Bacc()
    # x arrives pre-padded and chunked on host: per chunk 10 rows x 258
    # cols (reflect halo+pads materialized) so one contiguous DMA per
    # sample loads mains + halos + pads at once
    x_ext = nc.declare_dram_parameter("x", [S, NCH, C, RP, WP], F16, isOutput=False)
    sg_ext = nc.declare_dram_parameter("sigma", [S, T, H, W], F16, isOutput=False)
    id_ext = nc.declare_dram_parameter("ident", [128, 128], F16, isOutput=False)
    out_ext = nc.declare_dram_parameter("out", [S, C, H, W], F16, isOutput=True)

    with tile.TileContext(nc) as tc:
        with (
            tc.tile_pool(name="const", bufs=1) as cpool,
            tc.psum_pool(name="ps", bufs=1) as pspool,
            tc.tile_pool(name="io", bufs=2) as iopool,
            tc.tile_pool(name="pr", bufs=8) as prpool,
            tc.tile_pool(name="sc", bufs=2) as scpool,
        ):
            # stationary identity: matmul(I, prod) == copy-with-accumulate
            # into PSUM, so the PE does all tap + denominator summation
            ident = cpool.tile([128, 128], F16)
            nc.sync.dma_start(ident[:], id_ext[:])

            accs = pspool.tile([128, 2, C, BPX], F32)  # 6 banks
            dens = pspool.tile([128, 2, BPX], F32)  # 2 banks

            # one-half-deep software pipeline: evacuate half h while the
            # muls of half h+1 run, so neither DVE nor PE ever stalls
            pending = []

            def emit_mul(stripe, half, xt, st, j):
                r0 = HR * half
                taps = TAPS_H0 if half == 0 else TAPS_H1
                t = taps[j]
                di, dj = t // 3, t % 3
                with nc.allow_low_precision(reason="fp16 products"):
                    prod = prpool.tile([128, C, HR, W], F16, name="prod")
                    xs = xt[:, :, r0 + di : r0 + di + HR, dj : dj + W]
                    sg = st[:, t : t + 1, r0 : r0 + HR, :]
                    a, b = broadcast_tensor_aps(xs, sg)
                    nc.vector.tensor_mul(prod[:], a, b)
                return prod

            def emit_accmms(stripe, half, st, j, prod):
                first, last = j == 0, j == T - 1
                for blk in range(2):
                    for c in range(C):
                        nc.tensor.matmul(
                            accs[:, blk, c],
                            ident[:],
                            prod[:, c, BR * blk : BR * (blk + 1), :],
                            start=first,
                            stop=last,
                        )

            def emit_denmms(stripe, half, st, j):
                r0 = HR * half
                taps = TAPS_H0 if half == 0 else TAPS_H1
                t = taps[j]
                first, last = j == 0, j == T - 1
                for blk in range(2):
                    nc.tensor.matmul(
                        dens[:, blk],
                        ident[:],
                        st[:, t, r0 + BR * blk : r0 + BR * (blk + 1), :],
                        start=first,
                        stop=last,
                    )

            def emit_mms(stripe, half, st, j, prod):
                # last tap: den stops first so the reciprocal can fire early
                if j == T - 1:
                    emit_denmms(stripe, half, st, j)
                    emit_accmms(stripe, half, st, j, prod)
                else:
                    emit_accmms(stripe, half, st, j, prod)
                    emit_denmms(stripe, half, st, j)

            def emit_muls(stripe, half, xt, st, lo, hi):
                for j in range(lo, hi):
                    prod = emit_mul(stripe, half, xt, st, j)
                    emit_mms(stripe, half, st, j, prod)

            def emit_recip():
                inv = scpool.tile([128, 2, 1, BPX], F32, name="inv")
                # ~18 correct bits >> fp16 noise floor; den in (0, 9]
                nc.vector.reciprocal_approx_fast(inv[:, :, 0], dens[:])
                return inv

            def emit_evac():
                # ScalarE casts PSUM f32 -> SBUF fp16 (frees acc banks and
                # lets the DVE normalize run in fp16 2x mode); per-block
                # copies so the PE can re-enter block 0 sooner
                a16 = scpool.tile([128, 2, C, BPX], F16, name="a16")
                nc.scalar.copy(a16[:, 0], accs[:, 0])
                nc.scalar.copy(a16[:, 1], accs[:, 1])
                return a16

            def emit_norm(stripe, half, ot, inv, a16):
                r0 = HR * half
                otv = ot[:, :, r0 : r0 + HR, :].rearrange(
                    "p c (b r) w -> p b c (r w)", b=2
                )
                with nc.allow_low_precision(reason="fp16 out"):
                    if a16 is None:
                        # trailing half: normalize straight out of PSUM (1x,
                        # but skips the ScalarE evacuation hop on the final
                        # dependence chain)
                        a, b = broadcast_tensor_aps(accs[:], inv[:])
                        nc.vector.tensor_mul(otv, a, b)
                    else:
                        inv16 = scpool.tile([128, 2, 1, BPX], F16, name="inv16")
                        nc.scalar.copy(inv16[:, :, 0], inv[:, :, 0])
                        a, b = broadcast_tensor_aps(a16[:], inv16[:])
                        nc.vector.tensor_mul(otv, a, b)
                for k in range(SPS):
                    s = SPS * stripe + k
                    pb = NCH * k
                    oeng = nc.sync if k % 2 == 0 else nc.scalar
                    orr = out_ext[s].rearrange("c (n r) w -> n c r w", r=RC)
                    oeng.dma_start(
                        orr[:, :, r0 : r0 + HR, :],
                        ot[pb : pb + NCH, :, r0 : r0 + HR, :],
                    )

            def deng(k):
                return nc.sync if k % 2 == 0 else nc.scalar

            # All input DMAs for BOTH stripes are issued before any compute
            # or output DMA is emitted: the HWDGE rings drain FIFO, so an
            # output DMA (which waits on compute) emitted earlier would
            # head-of-line block the second stripe's prefetch.  Per-ring
            # order: center sigma + the first half's x rows first (the first
            # muls' inputs), then everything else in consumption order.
            # The SWDGE (gpsimd) ring is a third parallel channel for the
            # x loads of two samples (its descriptor generation all happens
            # up front while the DVE is idle, so no Q7/DVE contention).
            tiles = []
            for stripe in range(NSTRIPES):
                xt = iopool.tile([128, C, RP, WP], F16)
                st = iopool.tile([128, T, RC, W], F16)
                ot = iopool.tile([128, C, RC, W], F16)
                tiles.append((xt, st, ot))

                srs = []
                for k in range(SPS):
                    s = SPS * stripe + k
                    srs.append(sg_ext[s].rearrange("t (n r) w -> n t r w", r=RC))

                def sg_dma(k, tsl, half, eng):
                    pb = NCH * k
                    r0 = HR * half
                    eng.dma_start(
                        st[pb : pb + NCH, tsl, r0 : r0 + HR, :],
                        srs[k][:, tsl, r0 : r0 + HR, :],
                    )

                def x_dma(k, lo, hi, eng):
                    pb = NCH * k
                    s = SPS * stripe + k
                    eng.dma_start(
                        xt[pb : pb + NCH, :, lo:hi, :], x_ext[s][:, :, lo:hi, :]
                    )

                # half-granular sigma loads in exact consumption order; the
                # heavy groups of samples 2-3 ride the gpsimd (SWDGE) ring
                def third(k):
                    return nc.gpsimd if k >= 2 else deng(k)

                for k in range(SPS):
                    sg_dma(k, slice(4, 5), 0, deng(k))
                for k in range(SPS):
                    x_dma(k, 0, HR + 2, third(k))
                for k in range(SPS):
                    sg_dma(k, slice(3, 6, 2), 0, deng(k))
                for k in range(SPS):
                    sg_dma(k, slice(6, 9), 0, third(k))
                for k in range(SPS):
                    sg_dma(k, slice(0, 3), 0, third(k))
                for k in range(SPS):
                    x_dma(k, HR + 2, RP, third(k))
                for k in range(SPS):
                    sg_dma(k, slice(4, 5), 1, deng(k))
                for k in range(SPS):
                    sg_dma(k, slice(3, 6, 2), 1, deng(k))
                for k in range(SPS):
                    sg_dma(k, slice(0, 3), 1, third(k))
                for k in range(SPS):
                    sg_dma(k, slice(6, 9), 1, third(k))

            for stripe in range(NSTRIPES):
                xt, st, ot = tiles[stripe]
                for half in range(RC // HR):
                    # software pipeline: the first mul of this half runs on
                    # the DVE before the previous half's reciprocal, but its
                    # PE matmuls are emitted after it (program order defines
                    # the PSUM read-before-reuse dependency)
                    if pending:
                        # acc matmuls follow their muls immediately (they
                        # only need the ScalarE evacuation), while the den
                        # matmuls of the first three taps sit behind the
                        # previous half's reciprocal, giving the DVE three
                        # muls of runway to absorb the PE's catch-up lag
                        prods = []
                        for j in range(3):
                            prods.append(emit_mul(stripe, half, xt, st, j))
                            emit_accmms(stripe, half, st, j, prods[j])
                        (pstripe, phalf, pot, pa16) = pending.pop()
                        inv = emit_recip()
                        for j in range(3):
                            emit_denmms(stripe, half, st, j)
                        emit_muls(stripe, half, xt, st, 3, 7)
                        emit_norm(pstripe, phalf, pot, inv, pa16)
                        emit_muls(stripe, half, xt, st, 7, T)
                    else:
                        emit_muls(stripe, half, xt, st, 0, T)
                    final = stripe == NSTRIPES - 1 and half == RC // HR - 1
                    a16 = None if final else emit_evac()
                    pending.append((stripe, half, ot, a16))

            # trailing half: normalize straight from PSUM (a16=None)
            (pstripe, phalf, pot, pa16) = pending.pop()
            inv = emit_recip()
            emit_norm(pstripe, phalf, pot, inv, None)

    nc.finalize()
    return nc


_nc_cache = None


def _get_nc():
    global _nc_cache
    if _nc_cache is None:
        _nc_cache = build_nc()
    return _nc_cache


def _prep_x(x):
    """Reflect-pad and chunk x on host: [S,C,H,W] f32 ->
    [S,NCH,C,RP,WP] f16 with per-chunk halo rows/cols materialized."""
    xh = x.astype(np.float16)
    xp = np.pad(xh, ((0, 0), (0, 0), (1, 1), (1, 1)), mode="reflect")
    # windows: chunk n covers padded rows 8n..8n+9
    win = np.lib.stride_tricks.sliding_window_view(xp, RP, axis=2)
    win = win[:, :, ::RC, :, :]  # [S, C, NCH, WP, RP]
    win = np.transpose(win, (0, 2, 1, 4, 3))  # [S, NCH, C, RP, WP]
    return np.ascontiguousarray(win)


def _run(x, sigma, trace=False):
    xe = _prep_x(np.asarray(x))
    sigma = np.ascontiguousarray(sigma).astype(np.float16)
    ident = np.eye(128, dtype=np.float16)
    nc = _get_nc()
    in_maps = [
        {
            "x": xe[S * i : S * (i + 1)],
            "sigma": sigma[S * i : S * (i + 1)],
            "ident": ident,
        }
        for i in range(N_CORES)
    ]
    res = run_bass_kernel_spmd(nc, in_maps, list(range(N_CORES)), trace=trace)
    out = np.concatenate([res.results[i]["out"] for i in range(N_CORES)], axis=0)
    return out.astype(np.float32, copy=False), res


def kernel(x, sigma):
    out, _ = _run(x, sigma)
    return out


# revision 42
# speedup vs baseline: 1.0043x; 1.0043x over previous
import sys

import numpy as np

sys.path.insert(0, "/opt/trn_rl_repo")

from concourse import bacc, bass, tile  # noqa: E402,F401
from concourse import mybir  # noqa: E402
from concourse.bass import broadcast_tensor_aps  # noqa: E402
from concourse.bass_utils import run_bass_kernel_spmd  # noqa: E402

N_CORES = 8
S = 8  # samples per core
C = 3
T = 9
H = W = 256
RC = 8  # rows per chunk (one SBUF partition holds one chunk)
NCH = H // RC  # 32 chunks per sample
SPS = 128 // NCH  # 4 samples striped across the 128 partitions
NSTRIPES = S // SPS  # 2
RP = RC + 2  # rows per chunk incl halo (materialized on host)
WP = W + 2  # cols incl reflect pad (materialized on host)
HR = 4  # rows per mul tile (half a chunk)
BR = 2  # rows per PSUM block: 3ch acc + den = 4 banks, 2 blocks
BPX = BR * W  # 512 psum columns per block
F32 = mybir.dt.float32
F16 = mybir.dt.float16
# Per-half tap order: center tap first, then taps in merged-sigma-DMA
# arrival order {4}, {3,5}, {6,7,8}, {0,1,2}
TAPS_H0 = [4, 5, 3, 8, 7, 6, 2, 1, 0]
TAPS_H1 = [4, 5, 3, 2, 1, 0, 8, 7, 6]


def build_nc():
    nc = bacc.Bacc()
    # x arrives pre-padded and chunked on host: per chunk 10 rows x 258
    # cols (reflect halo+pads materialized) so one contiguous DMA per
    # sample loads mains + halos + pads at once
    x_ext = nc.declare_dram_parameter("x", [S, NCH, C, RP, WP], F16, isOutput=False)
    sg_ext = nc.declare_dram_parameter("sigma", [S, T, H, W], F16, isOutput=False)
    id_ext = nc.declare_dram_parameter("ident", [128, 128], F16, isOutput=False)
    out_ext = nc.declare_dram_parameter("out", [S, C, H, W], F16, isOutput=True)

    with tile.TileContext(nc) as tc:
        with (
            tc.tile_pool(name="const", bufs=1) as cpool,
            tc.psum_pool(name="ps", bufs=1) as pspool,
            tc.tile_pool(name="io", bufs=2) as iopool,
            tc.tile_pool(name="pr", bufs=8) as prpool,
            tc.tile_pool(name="sc", bufs=2) as scpool,
        ):
            # stationary identity: matmul(I, prod) == copy-with-accumulate
            # into PSUM, so the PE does all tap + denominator summation
            ident = cpool.tile([128, 128], F16)
            nc.sync.dma_start(ident[:], id_ext[:])

            accs = pspool.tile([128, 2, C, BPX], F32)  # 6 banks
            dens = pspool.tile([128, 2, BPX], F32)  # 2 banks

            # one-half-deep software pipeline: evacuate half h while the
            # muls of half h+1 run, so neither DVE nor PE ever stalls
            pending = []

            def emit_mul(stripe, half, xt, st, j):
                r0 = HR * half
                taps = TAPS_H0 if half == 0 else TAPS_H1
                t = taps[j]
                di, dj = t // 3, t % 3
                with nc.allow_low_precision(reason="fp16 products"):
                    prod = prpool.tile([128, C, HR, W], F16, name="prod")
                    xs = xt[:, :, r0 + di : r0 + di + HR, dj : dj + W]
                    sg = st[:, t : t + 1, r0 : r0 + HR, :]
                    a, b = broadcast_tensor_aps(xs, sg)
                    nc.vector.tensor_mul(prod[:], a, b)
                return prod

            def emit_accmms(stripe, half, st, j, prod):
                first, last = j == 0, j == T - 1
                for blk in range(2):
                    for c in range(C):
                        nc.tensor.matmul(
                            accs[:, blk, c],
                            ident[:],
                            prod[:, c, BR * blk : BR * (blk + 1), :],
                            start=first,
                            stop=last,
                        )

            def emit_denmms(stripe, half, st, j):
                r0 = HR * half
                taps = TAPS_H0 if half == 0 else TAPS_H1
                t = taps[j]
                first, last = j == 0, j == T - 1
                for blk in range(2):
                    nc.tensor.matmul(
                        dens[:, blk],
                        ident[:],
                        st[:, t, r0 + BR * blk : r0 + BR * (blk + 1), :],
                        start=first,
                        stop=last,
                    )

            def emit_mms(stripe, half, st, j, prod):
                # last tap: den stops first so the reciprocal can fire early
                if j == T - 1:
                    emit_denmms(stripe, half, st, j)
                    emit_accmms(stripe, half, st, j, prod)
                else:
                    emit_accmms(stripe, half, st, j, prod)
                    emit_denmms(stripe, half, st, j)

            def emit_muls(stripe, half, xt, st, lo, hi):
                for j in range(lo, hi):
                    prod = emit_mul(stripe, half, xt, st, j)
                    emit_mms(stripe, half, st, j, prod)

            def emit_recip():
                inv = scpool.tile([128, 2, 1, BPX], F32, name="inv")
                # ~18 correct bits >> fp16 noise floor; den in (0, 9]
                nc.vector.reciprocal_approx_fast(inv[:, :, 0], dens[:])
                return inv

            def emit_evac():
                # ScalarE casts PSUM f32 -> SBUF fp16 (frees acc banks and
                # lets the DVE normalize run in fp16 2x mode); per-block
                # copies so the PE can re-enter block 0 sooner
                a16 = scpool.tile([128, 2, C, BPX], F16, name="a16")
                nc.scalar.copy(a16[:, 0], accs[:, 0])
                nc.scalar.copy(a16[:, 1], accs[:, 1])
                return a16

            def emit_norm(stripe, half, ot, inv, a16):
                r0 = HR * half
                otv = ot[:, :, r0 : r0 + HR, :].rearrange(
                    "p c (b r) w -> p b c (r w)", b=2
                )
                with nc.allow_low_precision(reason="fp16 out"):
                    if a16 is None:
                        # trailing half: normalize straight out of PSUM (1x,
                        # but skips the ScalarE evacuation hop on the final
                        # dependence chain)
                        a, b = broadcast_tensor_aps(accs[:], inv[:])
                        nc.vector.tensor_mul(otv, a, b)
                    else:
                        inv16 = scpool.tile([128, 2, 1, BPX], F16, name="inv16")
                        nc.scalar.copy(inv16[:, :, 0], inv[:, :, 0])
                        a, b = broadcast_tensor_aps(a16[:], inv16[:])
                        nc.vector.tensor_mul(otv, a, b)
                for k in range(SPS):
                    s = SPS * stripe + k
                    pb = NCH * k
                    oeng = nc.sync if k % 2 == 0 else nc.scalar
                    orr = out_ext[s].rearrange("c (n r) w -> n c r w", r=RC)
                    oeng.dma_start(
                        orr[:, :, r0 : r0 + HR, :],
                        ot[pb : pb + NCH, :, r0 : r0 + HR, :],
                    )

            def deng(k):
                return nc.sync if k % 2 == 0 else nc.scalar

            # All input DMAs for BOTH stripes are issued before any compute
            # or output DMA is emitted: the HWDGE rings drain FIFO, so an
            # output DMA (which waits on compute) emitted earlier would
            # head-of-line block the second stripe's prefetch.  Per-ring
            # order: center sigma + the first half's x rows first (the first
            # muls' inputs), then everything else in consumption order.
            # The SWDGE (gpsimd) ring is a third parallel channel for the
            # x loads of two samples (its descriptor generation all happens
            # up front while the DVE is idle, so no Q7/DVE contention).
            tiles = []
            for stripe in range(NSTRIPES):
                xt = iopool.tile([128, C, RP, WP], F16)
                st = iopool.tile([128, T, RC, W], F16)
                ot = iopool.tile([128, C, RC, W], F16)
                tiles.append((xt, st, ot))

                srs = []
                for k in range(SPS):
                    s = SPS * stripe + k
                    srs.append(sg_ext[s].rearrange("t (n r) w -> n t r w", r=RC))

                def sg_dma(k, tsl, half, eng):
                    pb = NCH * k
                    r0 = HR * half
                    eng.dma_start(
                        st[pb : pb + NCH, tsl, r0 : r0 + HR, :],
                        srs[k][:, tsl, r0 : r0 + HR, :],
                    )

                def x_dma(k, lo, hi, eng):
                    pb = NCH * k
                    s = SPS * stripe + k
                    eng.dma_start(
                        xt[pb : pb + NCH, :, lo:hi, :], x_ext[s][:, :, lo:hi, :]
                    )

                # half-granular sigma loads in exact consumption order; the
                # heavy groups of samples 2-3 ride the gpsimd (SWDGE) ring
                def third(k):
                    return nc.gpsimd if k >= 2 else deng(k)

                for k in range(SPS):
                    sg_dma(k, slice(4, 5), 0, deng(k))
                for k in range(SPS):
                    x_dma(k, 0, HR + 2, third(k))
                for k in range(SPS):
                    sg_dma(k, slice(3, 6, 2), 0, deng(k))
                for k in range(SPS):
                    sg_dma(k, slice(6, 9), 0, third(k))
                for k in range(SPS):
                    sg_dma(k, slice(0, 3), 0, third(k))
                for k in range(SPS):
                    x_dma(k, HR + 2, RP, third(k))
                for k in range(SPS):
                    sg_dma(k, slice(4, 5), 1, deng(k))
                for k in range(SPS):
                    sg_dma(k, slice(3, 6, 2), 1, deng(k))
                for k in range(SPS):
                    sg_dma(k, slice(0, 3), 1, third(k))
                for k in range(SPS):
                    sg_dma(k, slice(6, 9), 1, third(k))

            for stripe in range(NSTRIPES):
                xt, st, ot = tiles[stripe]
                for half in range(RC // HR):
                    # software pipeline: the first mul of this half runs on
                    # the DVE before the previous half's reciprocal, but its
                    # PE matmuls are emitted after it (program order defines
                    # the PSUM read-before-reuse dependency)
                    if pending:
                        # acc matmuls follow their muls immediately (they
                        # only need the ScalarE evacuation), while the den
                        # matmuls of the first three taps sit behind the
                        # previous half's reciprocal, giving the DVE three
                        # muls of runway to absorb the PE's catch-up lag
                        prods = []
                        for j in range(3):
                            prods.append(emit_mul(stripe, half, xt, st, j))
                            emit_accmms(stripe, half, st, j, prods[j])
                        (pstripe, phalf, pot, pa16) = pending.pop()
                        inv = emit_recip()
                        for j in range(3):
                            emit_denmms(stripe, half, st, j)
                        emit_muls(stripe, half, xt, st, 3, 5)
                        emit_norm(pstripe, phalf, pot, inv, pa16)
                        emit_muls(stripe, half, xt, st, 5, T)
                    else:
                        emit_muls(stripe, half, xt, st, 0, T)
                    final = stripe == NSTRIPES - 1 and half == RC // HR - 1
                    a16 = None if final else emit_evac()
                    pending.append((stripe, half, ot, a16))

            # trailing half: normalize straight from PSUM (a16=None)
            (pstripe, phalf, pot, pa16) = pending.pop()
            inv = emit_recip()
            emit_norm(pstripe, phalf, pot, inv, None)

    nc.finalize()
    return nc


_nc_cache = None


def _get_nc():
    global _nc_cache
    if _nc_cache is None:
        _nc_cache = build_nc()
    return _nc_cache


def _prep_x(x):
    """Reflect-pad and chunk x on host: [S,C,H,W] f32 ->
    [S,NCH,C,RP,WP] f16 with per-chunk halo rows/cols materialized."""
    xh = x.astype(np.float16)
    xp = np.pad(xh, ((0, 0), (0, 0), (1, 1), (1, 1)), mode="reflect")
    # windows: chunk n covers padded rows 8n..8n+9
    win = np.lib.stride_tricks.sliding_window_view(xp, RP, axis=2)
    win = win[:, :, ::RC, :, :]  # [S, C, NCH, WP, RP]
    win = np.transpose(win, (0, 2, 1, 4, 3))  # [S, NCH, C, RP, WP]
    return np.ascontiguousarray(win)


def _run(x, sigma, trace=False):
    xe = _prep_x(np.asarray(x))
    sigma = np.ascontiguousarray(sigma).astype(np.float16)
    ident = np.eye(128, dtype=np.float16)
    nc = _get_nc()
    in_maps = [
        {
            "x": xe[S * i : S * (i + 1)],
            "sigma": sigma[S * i : S * (i + 1)],
            "ident": ident,
        }
        for i in range(N_CORES)
    ]
    res = run_bass_kernel_spmd(nc, in_maps, list(range(N_CORES)), trace=trace)
    out = np.concatenate([res.results[i]["out"] for i in range(N_CORES)], axis=0)
    return out.astype(np.float32, copy=False), res


def kernel(x, sigma):
    out, _ = _run(x, sigma)
    return out


# revision 43
# speedup vs baseline: 1.0393x; 1.0349x over previous
import sys

import numpy as np

sys.path.insert(0, "/opt/trn_rl_repo")

from concourse import bacc, bass, tile  # noqa: E402,F401
from concourse import mybir  # noqa: E402
from concourse.bass import broadcast_tensor_aps  # noqa: E402
from concourse.bass_utils import run_bass_kernel_spmd  # noqa: E402

N_CORES = 8
S = 8  # samples per core
C = 3
T = 9
H = W = 256
RC = 8  # rows per chunk (one SBUF partition holds one chunk)
NCH = H // RC  # 32 chunks per sample
SPS = 128 // NCH  # 4 samples striped across the 128 partitions
NSTRIPES = S // SPS  # 2
RP = RC + 2  # rows per chunk incl halo (materialized on host)
WP = W + 2  # cols incl reflect pad (materialized on host)
HR = 4  # rows per mul tile (half a chunk)
BR = 2  # rows per PSUM block: 3ch acc + den = 4 banks, 2 blocks
BPX = BR * W  # 512 psum columns per block
F32 = mybir.dt.float32
F16 = mybir.dt.float16
# Per-half tap order: center tap first, then taps in merged-sigma-DMA
# arrival order {4}, {3,5}, {6,7,8}, {0,1,2}
TAPS_H0 = [4, 5, 3, 8, 7, 6, 2, 1, 0]
TAPS_H1 = [4, 5, 3, 2, 1, 0, 8, 7, 6]


def build_nc():
    nc = bacc.Bacc()
    # x arrives pre-padded and chunked on host: per chunk 10 rows x 258
    # cols (reflect halo+pads materialized) so one contiguous DMA per
    # sample loads mains + halos + pads at once
    x_ext = nc.declare_dram_parameter("x", [S, NCH, C, RP, WP], F16, isOutput=False)
    sg_ext = nc.declare_dram_parameter("sigma", [S, T, H, W], F16, isOutput=False)
    id_ext = nc.declare_dram_parameter("ident", [128, 128], F16, isOutput=False)
    out_ext = nc.declare_dram_parameter("out", [S, C, H, W], F16, isOutput=True)

    with tile.TileContext(nc) as tc:
        with (
            tc.tile_pool(name="const", bufs=1) as cpool,
            tc.psum_pool(name="ps", bufs=1) as pspool,
            tc.tile_pool(name="io", bufs=2) as iopool,
            tc.tile_pool(name="pr", bufs=8) as prpool,
            tc.tile_pool(name="sc", bufs=2) as scpool,
        ):
            # stationary identity: matmul(I, prod) == copy-with-accumulate
            # into PSUM, so the PE does all tap + denominator summation
            ident = cpool.tile([128, 128], F16)
            nc.sync.dma_start(ident[:], id_ext[:])

            accs = pspool.tile([128, 2, C, BPX], F32)  # 6 banks
            dens = pspool.tile([128, 2, BPX], F32)  # 2 banks

            # one-half-deep software pipeline: evacuate half h while the
            # muls of half h+1 run, so neither DVE nor PE ever stalls
            pending = []

            def emit_mul(stripe, half, xt, st, j):
                r0 = HR * half
                taps = TAPS_H0 if half == 0 else TAPS_H1
                t = taps[j]
                di, dj = t // 3, t % 3
                with nc.allow_low_precision(reason="fp16 products"):
                    prod = prpool.tile([128, C, HR, W], F16, name="prod")
                    xs = xt[:, :, r0 + di : r0 + di + HR, dj : dj + W]
                    sg = st[:, t : t + 1, r0 : r0 + HR, :]
                    a, b = broadcast_tensor_aps(xs, sg)
                    nc.vector.tensor_mul(prod[:], a, b)
                return prod

            def emit_accmms(stripe, half, st, j, prod):
                first, last = j == 0, j == T - 1
                for blk in range(2):
                    for c in range(C):
                        mm = nc.tensor.matmul(
                            accs[:, blk, c],
                            ident[:],
                            prod[:, c, BR * blk : BR * (blk + 1), :],
                            start=first,
                            stop=last,
                        )
                        mm.ins.is_weight_onezero = True

            def emit_denmms(stripe, half, st, j):
                r0 = HR * half
                taps = TAPS_H0 if half == 0 else TAPS_H1
                t = taps[j]
                first, last = j == 0, j == T - 1
                for blk in range(2):
                    mm = nc.tensor.matmul(
                        dens[:, blk],
                        ident[:],
                        st[:, t, r0 + BR * blk : r0 + BR * (blk + 1), :],
                        start=first,
                        stop=last,
                    )
                    mm.ins.is_weight_onezero = True

            def emit_mms(stripe, half, st, j, prod):
                # last tap: den stops first so the reciprocal can fire early
                if j == T - 1:
                    emit_denmms(stripe, half, st, j)
                    emit_accmms(stripe, half, st, j, prod)
                else:
                    emit_accmms(stripe, half, st, j, prod)
                    emit_denmms(stripe, half, st, j)

            def emit_muls(stripe, half, xt, st, lo, hi):
                for j in range(lo, hi):
                    prod = emit_mul(stripe, half, xt, st, j)
                    emit_mms(stripe, half, st, j, prod)

            def emit_recip():
                inv = scpool.tile([128, 2, 1, BPX], F32, name="inv")
                # ~18 correct bits >> fp16 noise floor; den in (0, 9]
                nc.vector.reciprocal_approx_fast(inv[:, :, 0], dens[:])
                return inv

            def emit_evac():
                # ScalarE casts PSUM f32 -> SBUF fp16 (frees acc banks and
                # lets the DVE normalize run in fp16 2x mode); per-block
                # copies so the PE can re-enter block 0 sooner
                a16 = scpool.tile([128, 2, C, BPX], F16, name="a16")
                nc.scalar.copy(a16[:, 0], accs[:, 0])
                nc.scalar.copy(a16[:, 1], accs[:, 1])
                return a16

            def emit_norm(stripe, half, ot, inv, a16):
                r0 = HR * half
                otv = ot[:, :, r0 : r0 + HR, :].rearrange(
                    "p c (b r) w -> p b c (r w)", b=2
                )
                with nc.allow_low_precision(reason="fp16 out"):
                    if a16 is None:
                        # trailing half: normalize straight out of PSUM (1x,
                        # but skips the ScalarE evacuation hop on the final
                        # dependence chain)
                        a, b = broadcast_tensor_aps(accs[:], inv[:])
                        nc.vector.tensor_mul(otv, a, b)
                    else:
                        inv16 = scpool.tile([128, 2, 1, BPX], F16, name="inv16")
                        nc.scalar.copy(inv16[:, :, 0], inv[:, :, 0])
                        a, b = broadcast_tensor_aps(a16[:], inv16[:])
                        nc.vector.tensor_mul(otv, a, b)
                for k in range(SPS):
                    s = SPS * stripe + k
                    pb = NCH * k
                    oeng = nc.sync if k % 2 == 0 else nc.scalar
                    orr = out_ext[s].rearrange("c (n r) w -> n c r w", r=RC)
                    oeng.dma_start(
                        orr[:, :, r0 : r0 + HR, :],
                        ot[pb : pb + NCH, :, r0 : r0 + HR, :],
                    )

            def deng(k):
                return nc.sync if k % 2 == 0 else nc.scalar

            # All input DMAs for BOTH stripes are issued before any compute
            # or output DMA is emitted: the HWDGE rings drain FIFO, so an
            # output DMA (which waits on compute) emitted earlier would
            # head-of-line block the second stripe's prefetch.  Per-ring
            # order: center sigma + the first half's x rows first (the first
            # muls' inputs), then everything else in consumption order.
            # The SWDGE (gpsimd) ring is a third parallel channel for the
            # x loads of two samples (its descriptor generation all happens
            # up front while the DVE is idle, so no Q7/DVE contention).
            tiles = []
            for stripe in range(NSTRIPES):
                xt = iopool.tile([128, C, RP, WP], F16)
                st = iopool.tile([128, T, RC, W], F16)
                ot = iopool.tile([128, C, RC, W], F16)
                tiles.append((xt, st, ot))

                srs = []
                for k in range(SPS):
                    s = SPS * stripe + k
                    srs.append(sg_ext[s].rearrange("t (n r) w -> n t r w", r=RC))

                def sg_dma(k, tsl, half, eng):
                    pb = NCH * k
                    r0 = HR * half
                    eng.dma_start(
                        st[pb : pb + NCH, tsl, r0 : r0 + HR, :],
                        srs[k][:, tsl, r0 : r0 + HR, :],
                    )

                def x_dma(k, lo, hi, eng):
                    pb = NCH * k
                    s = SPS * stripe + k
                    eng.dma_start(
                        xt[pb : pb + NCH, :, lo:hi, :], x_ext[s][:, :, lo:hi, :]
                    )

                # half-granular sigma loads in exact consumption order; the
                # heavy groups of samples 2-3 ride the gpsimd (SWDGE) ring
                def third(k):
                    return nc.gpsimd if k >= 2 else deng(k)

                for k in range(SPS):
                    sg_dma(k, slice(4, 5), 0, deng(k))
                for k in range(SPS):
                    x_dma(k, 0, HR + 2, third(k))
                for k in range(SPS):
                    sg_dma(k, slice(3, 6, 2), 0, deng(k))
                for k in range(SPS):
                    sg_dma(k, slice(6, 9), 0, third(k))
                for k in range(SPS):
                    sg_dma(k, slice(0, 3), 0, third(k))
                for k in range(SPS):
                    x_dma(k, HR + 2, RP, third(k))
                for k in range(SPS):
                    sg_dma(k, slice(4, 5), 1, deng(k))
                for k in range(SPS):
                    sg_dma(k, slice(3, 6, 2), 1, deng(k))
                for k in range(SPS):
                    sg_dma(k, slice(0, 3), 1, third(k))
                for k in range(SPS):
                    sg_dma(k, slice(6, 9), 1, third(k))

            for stripe in range(NSTRIPES):
                xt, st, ot = tiles[stripe]
                for half in range(RC // HR):
                    # software pipeline: the first mul of this half runs on
                    # the DVE before the previous half's reciprocal, but its
                    # PE matmuls are emitted after it (program order defines
                    # the PSUM read-before-reuse dependency)
                    if pending:
                        # acc matmuls follow their muls immediately (they
                        # only need the ScalarE evacuation), while the den
                        # matmuls of the first three taps sit behind the
                        # previous half's reciprocal, giving the DVE three
                        # muls of runway to absorb the PE's catch-up lag
                        prods = []
                        for j in range(3):
                            prods.append(emit_mul(stripe, half, xt, st, j))
                            emit_accmms(stripe, half, st, j, prods[j])
                        (pstripe, phalf, pot, pa16) = pending.pop()
                        inv = emit_recip()
                        for j in range(3):
                            emit_denmms(stripe, half, st, j)
                        emit_muls(stripe, half, xt, st, 3, 5)
                        emit_norm(pstripe, phalf, pot, inv, pa16)
                        emit_muls(stripe, half, xt, st, 5, T)
                    else:
                        emit_muls(stripe, half, xt, st, 0, T)
                    final = stripe == NSTRIPES - 1 and half == RC // HR - 1
                    a16 = None if final else emit_evac()
                    pending.append((stripe, half, ot, a16))

            # trailing half: normalize straight from PSUM (a16=None)
            (pstripe, phalf, pot, pa16) = pending.pop()
            inv = emit_recip()
            emit_norm(pstripe, phalf, pot, inv, None)

    nc.finalize()
    return nc


_nc_cache = None


def _get_nc():
    global _nc_cache
    if _nc_cache is None:
        _nc_cache = build_nc()
    return _nc_cache


def _prep_x(x):
    """Reflect-pad and chunk x on host: [S,C,H,W] f32 ->
    [S,NCH,C,RP,WP] f16 with per-chunk halo rows/cols materialized."""
    xh = x.astype(np.float16)
    xp = np.pad(xh, ((0, 0), (0, 0), (1, 1), (1, 1)), mode="reflect")
    # windows: chunk n covers padded rows 8n..8n+9
    win = np.lib.stride_tricks.sliding_window_view(xp, RP, axis=2)
    win = win[:, :, ::RC, :, :]  # [S, C, NCH, WP, RP]
    win = np.transpose(win, (0, 2, 1, 4, 3))  # [S, NCH, C, RP, WP]
    return np.ascontiguousarray(win)


def _run(x, sigma, trace=False):
    xe = _prep_x(np.asarray(x))
    sigma = np.ascontiguousarray(sigma).astype(np.float16)
    ident = np.eye(128, dtype=np.float16)
    nc = _get_nc()
    in_maps = [
        {
            "x": xe[S * i : S * (i + 1)],
            "sigma": sigma[S * i : S * (i + 1)],
            "ident": ident,
        }
        for i in range(N_CORES)
    ]
    res = run_bass_kernel_spmd(nc, in_maps, list(range(N_CORES)), trace=trace)
    out = np.concatenate([res.results[i]["out"] for i in range(N_CORES)], axis=0)
    return out.astype(np.float32, copy=False), res


def kernel(x, sigma):
    out, _ = _run(x, sigma)
    return out
